# revision 19
# baseline (speedup 1.0000x reference)
"""LIF spike kernel (T=4 scan with threshold reset) on 8 TRN2 NeuronCores.

Recurrence per element (tau=1, thresh=1):
    s_t     = m_{t-1} + x_t
    spike_t = (s_t > 1)           -> output
    m_t     = s_t * (s_t <= 1)    -> threshold reset

Sharding: pure data-parallel over the batch axis (dim 1, 64 -> 8 per core).

Design (v2, custom-DVE):
  - Carry s_t (pre-reset membrane) instead of m_t: s_{t+1} = s_t*(s_t<=1)
    + x_{t+1} is ONE fused custom DVE instruction (LIF_STEP_ANT), so the
    serial recurrence costs 3 DVE passes instead of 6 (fp32 tensor ops
    run at 1 elem/lane/cycle regardless of fusion). Bit-exact: the mult
    is by an exact 0/1 mask and the add is a single IEEE fp32 add, same
    order as the reference.
  - Spikes leave the device packed 2-per-byte: y = (s_a>1)*2 + (s_b>1)
    (PACK_SPIKES2_ANT, int8 out straight from the DVE write port, values
    0..3 exact). Output wire traffic is 2 MiB/core vs 4 MiB for the int8
    sign planes and 16 MiB for f32. (s-1>0 <=> s>1 in fp32: Sterbenz on
    (1,2), sign-preserving rounding elsewhere.)
    One PAGED pack per chunk over [P,2,F] covers both spike pairs --
    pages pack (t0,t2),(t1,t3) because tile A holds (s0,s1) and tile B
    (s2,s3); the host decode unpicks that pairing. 4 DVE ops per chunk.
  - Per-column budget: wire (4*4+2)*128 B = 5.9 ns/col @ ~390 GB/s
    measured; DVE 5 cyc = 5.2 ns/col + ~156 ns/op overhead -> the two
    are rate-matched (both ~48 us/core); this output split is the joint
    optimum (packing more shifts the wall to DVE, less to the wire).
    ACT/PE/GPSIMD idle. ~7.5 us fixed framework postamble (full
    semaphore-file reset) + ~2.5 us preamble are unavoidable.
  - Loads: per-plane DMAs for the first two chunks (DVE starts after
    planes 0,1 of chunk0 land -- each DVE stall re-pays ~2 us of DMA
    completion-receipt latency, so the early ramp matters), then 2 MB+
    plane-pair DMAs (small DMAs degrade wire rate: 327 GB/s at 1 MB vs
    ~400 at 4 MB measured). Stores ride the scalar HWDGE ring. Measured
    dead ends: 7-chunk ramps (per-op overhead beats the stall savings),
    (x0,x2)/(x1,x3) pair loads (LIF1 then needs both chunk DMAs ->
    full-chunk wait granularity), one whole-chunk DMA (DVE waits 4 MB).
"""

import numpy as np

import concourse.bacc as bacc
import concourse.mybir as mybir
import concourse.tile as tile
from concourse import bass_utils
from concourse import dve_ops as DO
from concourse.dve_spec import Spec, Src0, Src1, One, C0, lower, _has_src1
from concourse.dve_uop import DveOpSpec

T = 4
B_FULL = 64
C, H, W = 128, 32, 32
N_CORES = 8
B_LOC = B_FULL // N_CORES            # 8
N = B_LOC * C * H * W                # 1048576 elements per core per timestep
P = 128                              # SBUF partitions
NP = N // P                          # 8192 elements per partition row

_F32 = mybir.dt.float32
_I8 = mybir.dt.int8

# chunk widths (elements per partition); sum must be NP. Wire-bound, so
# DVE has slack; small first chunk lands fast (DVE starts early), big
# middle chunks amortize per-op + per-DMA overhead, small last chunks
# keep the post-load DVE/store tail short.
FS = [512, 1024, 2048, 2048, 1792, 768]
assert sum(FS) == NP


def _register(name, spec):
    """Idempotently append a custom DveOp to the module registry, computing
    uops_sha at runtime (self-contained kernels can't check in pins)."""
    for o in DO.OPS:
        if o.name == name:
            return o
    row = DO._CUSTOM_DVE_ROW_BASE + len(DO.OPS)
    assert row < 0x20
    DO._SUB_OPCODE_FOR_NAME[name] = row
    shas = {}
    for ver in ("v3", "v4"):
        tmp = DveOpSpec(name=name, opcode=row, uops=lower(spec, ver=ver),
                        rd1_en=_has_src1(spec))
        shas[ver] = tmp.sha(ver)
    op = DO.DveOp(name, spec, subdim=False, uops_sha=shas)
    DO.OPS.append(op)
    DO.CUSTOM_DVE_SPECS[name] = spec
    return op


_s = Src0 * (Src0 <= One)
LIF_STEP = _register(
    "LIF_STEP_ANT",
    Spec(
        body=_s + Src1,
        reference=lambda in0, in1, c0, c1, c2: (
            in0 * (in0 <= 1.0) + in1
        ).astype(np.float32),
    ),
)
PACK2 = _register(
    "PACK_SPIKES2_ANT",
    Spec(
        body=(Src0 > One) * C0 + (Src1 > One),
        reference=lambda in0, in1, c0, c1, c2: (
            (in0 > 1.0).astype(np.float32) * c0 + (in1 > 1.0)
        ).astype(np.float32),
    ),
)

_nc_cache = None


def _build(fs=tuple(FS), bufs=4, plane_chunks=2, split_rings=False):
    nc = bacc.Bacc(
        "TRN2",
        target_bir_lowering=False,
        debug=False,
        enable_asserts=False,
    )
    x_d = nc.dram_tensor("x", [T, N], _F32, kind="ExternalInput").ap()
    y_d = nc.dram_tensor("y", [2, N], _I8, kind="ExternalOutput").ap()

    def xsl(base, f):
        # [P, T, f] view of all four timestep planes for this chunk
        return x_d[:, P * base : P * (base + f)].rearrange(
            "t (p f) -> p t f", p=P
        )

    def ysl(base, f):
        return y_d[:, P * base : P * (base + f)].rearrange(
            "i (p f) -> p i f", p=P
        )

    with tile.TileContext(nc) as tc:
        with (
            tc.tile_pool(name="xa", bufs=bufs) as pa,
            tc.tile_pool(name="xb", bufs=bufs) as pb,
            tc.tile_pool(name="yy", bufs=bufs) as yp,
        ):
            base = 0
            for j, F in enumerate(fs):
                # tile A holds planes (x0, x1) -> (s0, s1); tile B holds
                # (x2, x3) -> (s2, s3). The spike pack runs as ONE paged
                # op over [P,2,F]: y = (A>1)*2 + (B>1), whose pages pack
                # spike pairs (t0,t2) and (t1,t3) -- the host decode
                # just unpicks that pairing. 4 DVE ops per chunk, and
                # the LIF chain starts after tile A's DMA alone.
                a = pa.tile([P, 2, F], _F32, tag="a", name=f"a_{j}")
                b = pb.tile([P, 2, F], _F32, tag="b", name=f"b_{j}")
                src = xsl(base, F)
                if j < plane_chunks:
                    # per-plane loads in timestep order: the DVE chain
                    # starts as soon as planes 0,1 land. With
                    # split_rings, the b planes issue on the scalar
                    # HWDGE ring in parallel with a's on sync.
                    eng_b = nc.scalar if split_rings else nc.sync
                    nc.sync.dma_start(a[:, 0], src[:, 0])
                    nc.sync.dma_start(a[:, 1], src[:, 1])
                    eng_b.dma_start(b[:, 0], src[:, 2])
                    eng_b.dma_start(b[:, 1], src[:, 3])
                else:
                    # plane-pair DMAs: large transfers for wire rate;
                    # prefetch runs ahead of the DVE by now
                    nc.sync.dma_start(a[:], src[:, 0:2])
                    nc.sync.dma_start(b[:], src[:, 2:4])

                v = nc.vector
                y = yp.tile([P, 2, F], _I8, tag="y", name=f"y_{j}")
                v._custom_dve(LIF_STEP, out=a[:, 1], in0=a[:, 0], in1=a[:, 1])
                v._custom_dve(LIF_STEP, out=b[:, 0], in0=a[:, 1], in1=b[:, 0])
                v._custom_dve(LIF_STEP, out=b[:, 1], in0=b[:, 0], in1=b[:, 1])
                v._custom_dve(PACK2, out=y[:], in0=a[:], in1=b[:], s0=2.0)
                nc.scalar.dma_start(ysl(base, F), y[:])
                base += F

    nc.compile()
    return nc


def _get_nc():
    global _nc_cache
    if _nc_cache is None:
        _nc_cache = _build()
    return _nc_cache


def _run(x, **spmd_kwargs):
    x = np.asarray(x, dtype=np.float32)
    assert x.shape == (T, B_FULL, C, H, W), x.shape
    in_maps = [
        {
            "x": np.ascontiguousarray(
                x[:, c * B_LOC : (c + 1) * B_LOC]
            ).reshape(T, N)
        }
        for c in range(N_CORES)
    ]
    res = bass_utils.run_bass_kernel_spmd(
        _get_nc(), in_maps, core_ids=list(range(N_CORES)), **spmd_kwargs
    )
    out = np.empty((T, B_FULL, C, H, W), dtype=np.float32)
    sh = (B_LOC, C, H, W)
    for c in range(N_CORES):
        y = res.results[c]["y"]  # [2, N] int8, rows pack (t0,t2), (t1,t3)
        b = slice(c * B_LOC, (c + 1) * B_LOC)
        out[0, b] = ((y[0] >> 1) & 1).astype(np.float32).reshape(sh)
        out[2, b] = (y[0] & 1).astype(np.float32).reshape(sh)
        out[1, b] = ((y[1] >> 1) & 1).astype(np.float32).reshape(sh)
        out[3, b] = (y[1] & 1).astype(np.float32).reshape(sh)
    return out, res


def kernel(x):
    out, _ = _run(x)
    return out


# revision 20
# speedup vs baseline: 1.0849x; 1.0849x over previous
"""LIF spike kernel (T=4 scan with threshold reset) on 8 TRN2 NeuronCores.

Recurrence per element (tau=1, thresh=1):
    s_t     = m_{t-1} + x_t
    spike_t = (s_t > 1)           -> output
    m_t     = s_t * (s_t <= 1)    -> threshold reset

Sharding: pure data-parallel over the batch axis (dim 1, 64 -> 8 per core).

Design (final, custom-DVE; HW exec ~67-70 us/core in quiet windows vs
~75-82 us for the previous all-stock-DVE + ACT-sign baseline under the
same conditions -- ambient HBM-pair contention adds up to ~7 us run-to-
run, so compare medians of interleaved runs):
  - Carry s_t (pre-reset membrane) instead of m_t: s_{t+1} = s_t*(s_t<=1)
    + x_{t+1} is ONE fused custom DVE instruction (LIF_STEP_ANT), so the
    serial recurrence costs 3 DVE passes instead of 6 (fp32 tensor ops
    run at 1 elem/lane/cycle regardless of fusion). Bit-exact: the mult
    is by an exact 0/1 mask and the add is a single IEEE fp32 add, same
    order as the reference.
  - Spikes leave the device packed 2-per-byte: y = (s_a>1)*2 + (s_b>1)
    (PACK_SPIKES2_ANT, int8 out straight from the DVE write port, values
    0..3 exact). Output wire traffic is 2 MiB/core vs 4 MiB for the int8
    sign planes and 16 MiB for f32. (s-1>0 <=> s>1 in fp32: Sterbenz on
    (1,2), sign-preserving rounding elsewhere.)
    One PAGED pack per chunk over [P,2,F] covers both spike pairs --
    pages pack (t0,t2),(t1,t3) because tile A holds (s0,s1) and tile B
    (s2,s3); the host decode unpicks that pairing. 4 DVE ops per chunk.
  - Per-column budget: wire (4*4+2)*128 B = 5.9 ns/col @ ~390 GB/s
    measured; DVE 5 cyc = 5.2 ns/col + ~156 ns/op overhead -> the two
    are rate-matched (both ~48 us/core); this output split is the joint
    optimum (packing more shifts the wall to DVE, less to the wire).
    ACT/PE/GPSIMD idle. ~7.5 us fixed framework postamble (full
    semaphore-file reset) + ~2.5 us preamble are unavoidable.
  - Loads: per-plane DMAs for the first two chunks (DVE starts after
    planes 0,1 of chunk0 land -- each DVE stall re-pays ~2 us of DMA
    completion-receipt latency, so the early ramp matters), then 2 MB+
    plane-pair DMAs (small DMAs degrade wire rate: 327 GB/s at 1 MB vs
    ~400 at 4 MB measured). Stores ride the scalar HWDGE ring. Measured
    dead ends: 7-chunk ramps (per-op overhead beats the stall savings),
    (x0,x2)/(x1,x3) pair loads (LIF1 then needs both chunk DMAs ->
    full-chunk wait granularity), one whole-chunk DMA (DVE waits 4 MB).
"""

import numpy as np

import concourse.bacc as bacc
import concourse.mybir as mybir
import concourse.tile as tile
from concourse import bass_utils
from concourse import dve_ops as DO
from concourse.dve_spec import Spec, Src0, Src1, One, C0, lower, _has_src1
from concourse.dve_uop import DveOpSpec

T = 4
B_FULL = 64
C, H, W = 128, 32, 32
N_CORES = 8
B_LOC = B_FULL // N_CORES            # 8
N = B_LOC * C * H * W                # 1048576 elements per core per timestep
P = 128                              # SBUF partitions
NP = N // P                          # 8192 elements per partition row

_F32 = mybir.dt.float32
_I8 = mybir.dt.int8

# chunk widths (elements per partition); sum must be NP. Wire-bound, so
# DVE has slack; small first chunk lands fast (DVE starts early), big
# middle chunks amortize per-op + per-DMA overhead, small last chunks
# keep the post-load DVE/store tail short.
FS = [512, 1024, 2048, 2048, 1792, 768]
assert sum(FS) == NP


def _register(name, spec):
    """Idempotently append a custom DveOp to the module registry, computing
    uops_sha at runtime (self-contained kernels can't check in pins)."""
    for o in DO.OPS:
        if o.name == name:
            return o
    row = DO._CUSTOM_DVE_ROW_BASE + len(DO.OPS)
    assert row < 0x20
    DO._SUB_OPCODE_FOR_NAME[name] = row
    shas = {}
    for ver in ("v3", "v4"):
        tmp = DveOpSpec(name=name, opcode=row, uops=lower(spec, ver=ver),
                        rd1_en=_has_src1(spec))
        shas[ver] = tmp.sha(ver)
    op = DO.DveOp(name, spec, subdim=False, uops_sha=shas)
    DO.OPS.append(op)
    DO.CUSTOM_DVE_SPECS[name] = spec
    return op


_s = Src0 * (Src0 <= One)
LIF_STEP = _register(
    "LIF_STEP_ANT",
    Spec(
        body=_s + Src1,
        reference=lambda in0, in1, c0, c1, c2: (
            in0 * (in0 <= 1.0) + in1
        ).astype(np.float32),
    ),
)
PACK2 = _register(
    "PACK_SPIKES2_ANT",
    Spec(
        body=(Src0 > One) * C0 + (Src1 > One),
        reference=lambda in0, in1, c0, c1, c2: (
            (in0 > 1.0).astype(np.float32) * c0 + (in1 > 1.0)
        ).astype(np.float32),
    ),
)

_nc_cache = None


def _build(fs=tuple(FS), bufs=4, plane_chunks=2, split_rings=False):
    nc = bacc.Bacc(
        "TRN2",
        target_bir_lowering=False,
        debug=False,
        enable_asserts=False,
    )
    x_d = nc.dram_tensor("x", [T, N], _F32, kind="ExternalInput").ap()
    y_d = nc.dram_tensor("y", [2, N], _I8, kind="ExternalOutput").ap()

    def xsl(base, f):
        # [P, T, f] view of all four timestep planes for this chunk
        return x_d[:, P * base : P * (base + f)].rearrange(
            "t (p f) -> p t f", p=P
        )

    def ysl(base, f):
        return y_d[:, P * base : P * (base + f)].rearrange(
            "i (p f) -> p i f", p=P
        )

    with tile.TileContext(nc) as tc:
        with (
            tc.tile_pool(name="xa", bufs=bufs) as pa,
            tc.tile_pool(name="xb", bufs=bufs) as pb,
            tc.tile_pool(name="yy", bufs=bufs) as yp,
        ):
            base = 0
            for j, F in enumerate(fs):
                # tile A holds planes (x0, x1) -> (s0, s1); tile B holds
                # (x2, x3) -> (s2, s3). The spike pack runs as ONE paged
                # op over [P,2,F]: y = (A>1)*2 + (B>1), whose pages pack
                # spike pairs (t0,t2) and (t1,t3) -- the host decode
                # just unpicks that pairing. 4 DVE ops per chunk, and
                # the LIF chain starts after tile A's DMA alone.
                a = pa.tile([P, 2, F], _F32, tag="a", name=f"a_{j}")
                b = pb.tile([P, 2, F], _F32, tag="b", name=f"b_{j}")
                src = xsl(base, F)
                if j < plane_chunks:
                    # per-plane loads in timestep order: the DVE chain
                    # starts as soon as planes 0,1 land. With
                    # split_rings, the b planes issue on the scalar
                    # HWDGE ring in parallel with a's on sync.
                    eng_b = nc.scalar if split_rings else nc.sync
                    nc.sync.dma_start(a[:, 0], src[:, 0])
                    nc.sync.dma_start(a[:, 1], src[:, 1])
                    eng_b.dma_start(b[:, 0], src[:, 2])
                    eng_b.dma_start(b[:, 1], src[:, 3])
                else:
                    # plane-pair DMAs: large transfers for wire rate;
                    # prefetch runs ahead of the DVE by now
                    nc.sync.dma_start(a[:], src[:, 0:2])
                    nc.sync.dma_start(b[:], src[:, 2:4])

                v = nc.vector
                y = yp.tile([P, 2, F], _I8, tag="y", name=f"y_{j}")
                v._custom_dve(LIF_STEP, out=a[:, 1], in0=a[:, 0], in1=a[:, 1])
                v._custom_dve(LIF_STEP, out=b[:, 0], in0=a[:, 1], in1=b[:, 0])
                v._custom_dve(LIF_STEP, out=b[:, 1], in0=b[:, 0], in1=b[:, 1])
                v._custom_dve(PACK2, out=y[:], in0=a[:], in1=b[:], s0=2.0)
                nc.scalar.dma_start(ysl(base, F), y[:])
                base += F

    nc.compile()
    return nc


def _get_nc():
    global _nc_cache
    if _nc_cache is None:
        _nc_cache = _build()
    return _nc_cache


def _run(x, **spmd_kwargs):
    x = np.asarray(x, dtype=np.float32)
    assert x.shape == (T, B_FULL, C, H, W), x.shape
    in_maps = [
        {
            "x": np.ascontiguousarray(
                x[:, c * B_LOC : (c + 1) * B_LOC]
            ).reshape(T, N)
        }
        for c in range(N_CORES)
    ]
    res = bass_utils.run_bass_kernel_spmd(
        _get_nc(), in_maps, core_ids=list(range(N_CORES)), **spmd_kwargs
    )
    out = np.empty((T, B_FULL, C, H, W), dtype=np.float32)
    sh = (B_LOC, C, H, W)
    for c in range(N_CORES):
        y = res.results[c]["y"]  # [2, N] int8, rows pack (t0,t2), (t1,t3)
        b = slice(c * B_LOC, (c + 1) * B_LOC)
        out[0, b] = ((y[0] >> 1) & 1).astype(np.float32).reshape(sh)
        out[2, b] = (y[0] & 1).astype(np.float32).reshape(sh)
        out[1, b] = ((y[1] >> 1) & 1).astype(np.float32).reshape(sh)
        out[3, b] = (y[1] & 1).astype(np.float32).reshape(sh)
    return out, res


def kernel(x):
    out, _ = _run(x)
    return out


# revision 25
# speedup vs baseline: 1.1777x; 1.0855x over previous
"""LIF spike kernel (T=4 scan with threshold reset) on 8 TRN2 NeuronCores.

Recurrence per element (tau=1, thresh=1):
    s_t     = m_{t-1} + x_t
    spike_t = (s_t > 1)           -> output
    m_t     = s_t * (s_t <= 1)    -> threshold reset

Sharding: pure data-parallel over the batch axis (dim 1, 64 -> 8 per core).

Design (final, custom-DVE; HW exec ~67-70 us/core in quiet windows vs
~75-82 us for the previous all-stock-DVE + ACT-sign baseline under the
same conditions -- ambient HBM-pair contention adds up to ~7 us run-to-
run, so compare medians of interleaved runs):
  - Carry s_t (pre-reset membrane) instead of m_t: s_{t+1} = s_t*(s_t<=1)
    + x_{t+1} is ONE fused custom DVE instruction (LIF_STEP_ANT), so the
    serial recurrence costs 3 DVE passes instead of 6 (fp32 tensor ops
    run at 1 elem/lane/cycle regardless of fusion). Bit-exact: the mult
    is by an exact 0/1 mask and the add is a single IEEE fp32 add, same
    order as the reference.
  - Spikes leave the device packed 2-per-byte: y = (s_a>1)*2 + (s_b>1)
    (PACK_SPIKES2_ANT, int8 out straight from the DVE write port, values
    0..3 exact). Output wire traffic is 2 MiB/core vs 4 MiB for the int8
    sign planes and 16 MiB for f32. (s-1>0 <=> s>1 in fp32: Sterbenz on
    (1,2), sign-preserving rounding elsewhere.)
    One PAGED pack per chunk over [P,2,F] covers both spike pairs --
    pages pack (t0,t2),(t1,t3) because tile A holds (s0,s1) and tile B
    (s2,s3); the host decode unpicks that pairing. 4 DVE ops per chunk.
  - Per-column budget: wire (4*4+2)*128 B = 5.9 ns/col @ ~390 GB/s
    measured; DVE 5 cyc = 5.2 ns/col + ~156 ns/op overhead -> the two
    are rate-matched (both ~48 us/core); this output split is the joint
    optimum (packing more shifts the wall to DVE, less to the wire).
    ACT/PE/GPSIMD idle. ~7.5 us fixed framework postamble (full
    semaphore-file reset) + ~2.5 us preamble are unavoidable.
  - Loads: per-plane DMAs for the first two chunks (DVE starts after
    planes 0,1 of chunk0 land -- each DVE stall re-pays ~2 us of DMA
    completion-receipt latency, so the early ramp matters), then 2 MB+
    plane-pair DMAs (small DMAs degrade wire rate: 327 GB/s at 1 MB vs
    ~400 at 4 MB measured). Stores ride the scalar HWDGE ring. The last
    chunk packs per-row (same bit pairing) so its first store overlaps
    LIF3 and the final store+receipt that gates the postamble barrier is
    half-size (A/B: median -1.7 us, fast-mode rate 4/8 vs 1/8).
    Measured dead ends: 7-chunk ramps (per-op overhead beats the stall
    savings), (x0,x2)/(x1,x3) pair loads (LIF1 then needs both chunk
    DMAs -> full-chunk wait granularity), whole-chunk 4 MB DMAs (DVE
    completion waits too coarse), half-plane chunk0 loads, ring-split
    early loads (all at or below the +-3 us ambient noise floor).
"""

import numpy as np

import concourse.bacc as bacc
import concourse.mybir as mybir
import concourse.tile as tile
from concourse import bass_utils
from concourse import dve_ops as DO
from concourse.dve_spec import Spec, Src0, Src1, One, C0, lower, _has_src1
from concourse.dve_uop import DveOpSpec

T = 4
B_FULL = 64
C, H, W = 128, 32, 32
N_CORES = 8
B_LOC = B_FULL // N_CORES            # 8
N = B_LOC * C * H * W                # 1048576 elements per core per timestep
P = 128                              # SBUF partitions
NP = N // P                          # 8192 elements per partition row

_F32 = mybir.dt.float32
_I8 = mybir.dt.int8

# chunk widths (elements per partition); sum must be NP. Wire-bound, so
# DVE has slack; small first chunk lands fast (DVE starts early), big
# middle chunks amortize per-op + per-DMA overhead, small last chunks
# keep the post-load DVE/store tail short.
FS = [512, 1024, 2048, 2048, 1792, 768]
assert sum(FS) == NP


def _register(name, spec):
    """Idempotently append a custom DveOp to the module registry, computing
    uops_sha at runtime (self-contained kernels can't check in pins)."""
    for o in DO.OPS:
        if o.name == name:
            return o
    row = DO._CUSTOM_DVE_ROW_BASE + len(DO.OPS)
    assert row < 0x20
    DO._SUB_OPCODE_FOR_NAME[name] = row
    shas = {}
    for ver in ("v3", "v4"):
        tmp = DveOpSpec(name=name, opcode=row, uops=lower(spec, ver=ver),
                        rd1_en=_has_src1(spec))
        shas[ver] = tmp.sha(ver)
    op = DO.DveOp(name, spec, subdim=False, uops_sha=shas)
    DO.OPS.append(op)
    DO.CUSTOM_DVE_SPECS[name] = spec
    return op


_s = Src0 * (Src0 <= One)
LIF_STEP = _register(
    "LIF_STEP_ANT",
    Spec(
        body=_s + Src1,
        reference=lambda in0, in1, c0, c1, c2: (
            in0 * (in0 <= 1.0) + in1
        ).astype(np.float32),
    ),
)
PACK2 = _register(
    "PACK_SPIKES2_ANT",
    Spec(
        body=(Src0 > One) * C0 + (Src1 > One),
        reference=lambda in0, in1, c0, c1, c2: (
            (in0 > 1.0).astype(np.float32) * c0 + (in1 > 1.0)
        ).astype(np.float32),
    ),
)

_nc_cache = None


def _build(fs=tuple(FS), bufs=4, plane_chunks=2, split_rings=False,
           whole_mid=False, split_tail=True, half_c0=False):
    nc = bacc.Bacc(
        "TRN2",
        target_bir_lowering=False,
        debug=False,
        enable_asserts=False,
    )
    x_d = nc.dram_tensor("x", [T, N], _F32, kind="ExternalInput").ap()
    y_d = nc.dram_tensor("y", [2, N], _I8, kind="ExternalOutput").ap()

    def xsl(base, f):
        # [P, T, f] view of all four timestep planes for this chunk
        return x_d[:, P * base : P * (base + f)].rearrange(
            "t (p f) -> p t f", p=P
        )

    def ysl(base, f):
        return y_d[:, P * base : P * (base + f)].rearrange(
            "i (p f) -> p i f", p=P
        )

    with tile.TileContext(nc) as tc:
        with (
            tc.tile_pool(name="xa", bufs=bufs) as pa,
            tc.tile_pool(name="xb", bufs=bufs) as pb,
            tc.tile_pool(name="yy", bufs=bufs) as yp,
        ):
            base = 0
            nchunk = len(fs)
            for j, F in enumerate(fs):
                # tile A holds planes (x0, x1) -> (s0, s1); tile B holds
                # (x2, x3) -> (s2, s3). The spike pack runs as ONE paged
                # op over [P,2,F]: y = (A>1)*2 + (B>1), whose pages pack
                # spike pairs (t0,t2) and (t1,t3) -- the host decode
                # just unpicks that pairing. 4 DVE ops per chunk, and
                # the LIF chain starts after tile A's DMA alone.
                src = xsl(base, F)
                if whole_mid and j >= plane_chunks:
                    # one 4-plane DMA per chunk: biggest transfers;
                    # prefetch depth hides the coarser completion
                    # granularity by now
                    xt = pa.tile([P, 4, F], _F32, tag="w", name=f"w_{j}")
                    nc.sync.dma_start(xt[:], src)
                    A0, A1, B0, B1 = (xt[:, t] for t in range(4))
                    pkA, pkB = xt[:, 0:2], xt[:, 2:4]
                else:
                    a = pa.tile([P, 2, F], _F32, tag="a", name=f"a_{j}")
                    b = pb.tile([P, 2, F], _F32, tag="b", name=f"b_{j}")
                    A0, A1, B0, B1 = a[:, 0], a[:, 1], b[:, 0], b[:, 1]
                    pkA, pkB = a[:], b[:]
                    if j < plane_chunks:
                        # per-plane loads in timestep order: the DVE
                        # chain starts as soon as planes 0,1 land. With
                        # split_rings, the b planes issue on the scalar
                        # HWDGE ring in parallel with a's on sync.
                        eng_b = nc.scalar if split_rings else nc.sync
                        if j == 0 and half_c0:
                            h = F // 2
                            for t in range(2):
                                nc.sync.dma_start(a[:, t, :h], src[:, t, :h])
                                nc.sync.dma_start(a[:, t, h:], src[:, t, h:])
                        else:
                            nc.sync.dma_start(a[:, 0], src[:, 0])
                            nc.sync.dma_start(a[:, 1], src[:, 1])
                        eng_b.dma_start(b[:, 0], src[:, 2])
                        eng_b.dma_start(b[:, 1], src[:, 3])
                    else:
                        # plane-pair DMAs: large transfers for wire
                        # rate; prefetch runs ahead of the DVE by now
                        nc.sync.dma_start(a[:], src[:, 0:2])
                        nc.sync.dma_start(b[:], src[:, 2:4])

                v = nc.vector
                y = yp.tile([P, 2, F], _I8, tag="y", name=f"y_{j}")
                v._custom_dve(LIF_STEP, out=A1, in0=A0, in1=A1)
                v._custom_dve(LIF_STEP, out=B0, in0=A1, in1=B0)
                if split_tail and j == nchunk - 1:
                    # last chunk: pack row0 (spikes t0,t2 -- same bit
                    # pairing as the paged op) before LIF3 so its store
                    # overlaps the rest; the final store is half-size
                    v._custom_dve(PACK2, out=y[:, 0], in0=A0, in1=B0,
                                  s0=2.0)
                    nc.scalar.dma_start(ysl(base, F)[:, 0], y[:, 0])
                    v._custom_dve(LIF_STEP, out=B1, in0=B0, in1=B1)
                    v._custom_dve(PACK2, out=y[:, 1], in0=A1, in1=B1,
                                  s0=2.0)
                    nc.scalar.dma_start(ysl(base, F)[:, 1], y[:, 1])
                else:
                    v._custom_dve(LIF_STEP, out=B1, in0=B0, in1=B1)
                    v._custom_dve(PACK2, out=y[:], in0=pkA, in1=pkB, s0=2.0)
                    nc.scalar.dma_start(ysl(base, F), y[:])
                base += F

    nc.compile()
    return nc


def _get_nc():
    global _nc_cache
    if _nc_cache is None:
        _nc_cache = _build()
    return _nc_cache


def _run(x, **spmd_kwargs):
    x = np.asarray(x, dtype=np.float32)
    assert x.shape == (T, B_FULL, C, H, W), x.shape
    in_maps = [
        {
            "x": np.ascontiguousarray(
                x[:, c * B_LOC : (c + 1) * B_LOC]
            ).reshape(T, N)
        }
        for c in range(N_CORES)
    ]
    res = bass_utils.run_bass_kernel_spmd(
        _get_nc(), in_maps, core_ids=list(range(N_CORES)), **spmd_kwargs
    )
    out = np.empty((T, B_FULL, C, H, W), dtype=np.float32)
    sh = (B_LOC, C, H, W)
    for c in range(N_CORES):
        y = res.results[c]["y"]  # [2, N] int8, rows pack (t0,t2), (t1,t3)
        b = slice(c * B_LOC, (c + 1) * B_LOC)
        out[0, b] = ((y[0] >> 1) & 1).astype(np.float32).reshape(sh)
        out[2, b] = (y[0] & 1).astype(np.float32).reshape(sh)
        out[1, b] = ((y[1] >> 1) & 1).astype(np.float32).reshape(sh)
        out[3, b] = (y[1] & 1).astype(np.float32).reshape(sh)
    return out, res


def kernel(x):
    out, _ = _run(x)
    return out
